# revision 1
# baseline (speedup 1.0000x reference)
"""Trainium2 Bass kernel for the DN (topk_masking) module.

Computes, for each batch row b:
    winner_b = argmax_y ( (x_b . x2y_w[y]) / ||x2y_w[y]|| ) * [age_y >= 1]
    output[b]  = y2z_w[:, winner_b]
    z_pre[b]   = argmax_z y2z_w[z, winner_b]
(l2-normalizing x does not change the argmax, so it is skipped.)

Strategy (8 NeuronCores, y-dim sharded, 8192 y per core):
  phase 1: stream the age-masked bf16 copy of W.T through the TensorEngine
           against the stationary bf16 x.T -> scores [256 b, 8192 y] per core.
           Per-region top-8 (max8 + find_index8) runs as each score region
           completes, hiding under the stream; a small on-device merge picks
           each b-chunk's global top-4 candidate indices.
  phase 2: one indirect DMA per candidate column gathers the exact fp32 W rows;
           exact fp32 dot (DVE) and row norm (ScalarE square+accum) per
           candidate are written out.
  host:    merges the 8 cores' candidates exactly (value desc, index asc
           tie-break, masked rows exactly 0), gathers y2z_w columns.
"""
import numpy as np
import ml_dtypes
from contextlib import ExitStack

import concourse.bass as bass
import concourse.tile as tile
from concourse import bacc, mybir
from concourse.bass_utils import run_bass_kernel_spmd
from concourse.tile import add_dep_helper

N_CORES = 8
Y = 65536
YL = Y // N_CORES      # 8192
X = 1024
B = 256
Z = 100
SLAB = 512
NSLAB = YL // SLAB     # 16
NCAND = 4
REG_SLABS = [0, 8, 12, 15, 16]   # score regions, in slabs
NREG = len(REG_SLABS) - 1
BF = ml_dtypes.bfloat16

_nc_cache = [None]


def _build():
    nc = bacc.Bacc("TRN2", target_bir_lowering=False, debug=False,
                   num_devices=N_CORES)
    f32, bf16, u32 = mybir.dt.float32, mybir.dt.bfloat16, mybir.dt.uint32
    MUL = mybir.AluOpType.mult
    EQ = mybir.AluOpType.is_equal

    xTb = nc.dram_tensor("xTb", [X, B], bf16, kind="ExternalInput").ap()
    xf = nc.dram_tensor("xf", [B, X], f32, kind="ExternalInput").ap()
    wTb = nc.dram_tensor("wTb", [X, YL], bf16, kind="ExternalInput").ap()
    wf = nc.dram_tensor("wf", [YL, X], f32, kind="ExternalInput").ap()
    iota_in = nc.dram_tensor("iota", [128, NREG * 8], f32,
                             kind="ExternalInput").ap()
    cidx = nc.dram_tensor("cidx", [2, 128, NCAND], u32,
                          kind="ExternalOutput").ap()
    cp1 = nc.dram_tensor("cp1", [2, 128, 8], bf16, kind="ExternalOutput").ap()
    cdot = nc.dram_tensor("cdot", [2, 128, NCAND], f32,
                          kind="ExternalOutput").ap()
    cnrm = nc.dram_tensor("cnrm", [2, 128, NCAND], f32,
                          kind="ExternalOutput").ap()

    with tile.TileContext(nc) as tc, ExitStack() as ctx:
        const = ctx.enter_context(tc.tile_pool(name="const", bufs=1))
        wtp = ctx.enter_context(tc.tile_pool(name="wt", bufs=3))
        psp = ctx.enter_context(tc.tile_pool(name="ps", bufs=6, space="PSUM"))
        scp = ctx.enter_context(tc.tile_pool(name="sc", bufs=1))
        smal = ctx.enter_context(tc.tile_pool(name="sm", bufs=1))
        gthp = ctx.enter_context(tc.tile_pool(name="gth", bufs=2))
        scrp = ctx.enter_context(tc.tile_pool(name="scr", bufs=2))

        xtb = const.tile([128, 8, B], bf16)
        nc.sync.dma_start(xtb[:], xTb.rearrange("(c p) b -> p c b", p=128))
        xft = const.tile([128, 2, X], f32)
        nc.scalar.dma_start(xft[:], xf.rearrange("(k p) x -> p k x", p=128))
        iota = const.tile([128, NREG * 8], f32)
        nc.scalar.dma_start(iota[:], iota_in[:])

        sc = [scp.tile([128, YL], bf16, tag=f"sc{k}", name=f"sc{k}")
              for k in range(2)]
        vflat = [smal.tile([128, NREG * 8], bf16, tag=f"vf{k}", name=f"vf{k}")
                 for k in range(2)]
        iflat = [smal.tile([128, NREG * 8], f32, tag=f"if{k}", name=f"if{k}")
                 for k in range(2)]
        last_wt_dma = [None]

        def do_slab(s):
            wt = wtp.tile([128, 8, SLAB], bf16, name="wt")
            eng = nc.scalar if s == 0 else nc.sync
            d = eng.dma_start(
                wt[:],
                wTb[:, s * SLAB:(s + 1) * SLAB]
                .rearrange("(c p) y -> p c y", p=128))
            last_wt_dma[0] = d
            for k in range(2):
                ps = psp.tile([128, SLAB], mybir.dt.float32, name="ps")
                for c in range(8):
                    nc.tensor.matmul(
                        ps[:], lhsT=xtb[:, c, k * 128:(k + 1) * 128],
                        rhs=wt[:, c, :], start=(c == 0), stop=(c == 7))
                nc.scalar.copy(sc[k][:, s * SLAB:(s + 1) * SLAB], ps[:])

        def region_topk(k, r):
            lo, hi = REG_SLABS[r] * SLAB, REG_SLABS[r + 1] * SLAB
            nc.vector.max(vflat[k][:, r * 8:(r + 1) * 8], sc[k][:, lo:hi])
            mi = smal.tile([128, 8], mybir.dt.uint32,
                           tag=f"mi{k}{r}", name=f"mi{k}{r}")
            nc.vector.max_index(mi[:], vflat[k][:, r * 8:(r + 1) * 8],
                                sc[k][:, lo:hi])
            nc.vector.tensor_scalar_add(iflat[k][:, r * 8:(r + 1) * 8],
                                        mi[:], lo)

        for s in range(NSLAB):
            do_slab(s)
            for r in range(NREG - 1):
                if s == REG_SLABS[r + 1] - 1:
                    region_topk(0, r)
                    region_topk(1, r)
        region_topk(0, NREG - 1)
        region_topk(1, NREG - 1)

        for k in range(2):
            gmax = smal.tile([128, 8], mybir.dt.bfloat16,
                             tag=f"gm{k}", name=f"gm{k}")
            nc.vector.max(gmax[:], vflat[k][:])
            gpos = smal.tile([128, 8], mybir.dt.uint32,
                             tag=f"gp{k}", name=f"gp{k}")
            nc.vector.max_index(gpos[:], gmax[:], vflat[k][:])
            nc.sync.dma_start(cp1[k], gmax[:])
            gposf = smal.tile([128, 8], mybir.dt.float32,
                              tag=f"gpf{k}", name=f"gpf{k}")
            nc.vector.tensor_copy(gposf[:], gpos[:])
            cif = smal.tile([128, NCAND], mybir.dt.float32,
                            tag=f"cif{k}", name=f"cif{k}")
            pscr = scrp.tile([128, NREG * 8], mybir.dt.float32,
                             tag="pscr", name="pscr")
            for j in range(NCAND):
                # cif[:, j] = sum_t (iota[t] == gpos[j]) * iflat[t]
                nc.vector.scalar_tensor_tensor(
                    out=pscr[:], in0=iota[:], scalar=gposf[:, j:j + 1],
                    in1=iflat[k][:], op0=EQ, op1=MUL,
                    accum_out=cif[:, j:j + 1])
            ci = smal.tile([128, NCAND], mybir.dt.uint32,
                           tag=f"ci{k}", name=f"ci{k}")
            nc.vector.tensor_copy(ci[:], cif[:])
            nc.sync.dma_start(cidx[k], ci[:])

            dt = smal.tile([128, NCAND], mybir.dt.float32,
                           tag=f"dt{k}", name=f"dt{k}")
            nm = smal.tile([128, NCAND], mybir.dt.float32,
                           tag=f"nm{k}", name=f"nm{k}")
            for j in range(NCAND):
                gthj = gthp.tile([128, X], mybir.dt.float32,
                                 tag=f"gth{j}", name=f"gth{j}")
                g = nc.gpsimd.indirect_dma_start(
                    out=gthj[:], out_offset=None, in_=wf[:],
                    in_offset=bass.IndirectOffsetOnAxis(
                        ap=ci[:, j:j + 1], axis=0))
                if last_wt_dma[0] is not None:
                    add_dep_helper(g.ins, last_wt_dma[0].ins,
                                   reason="defer gather past stream")
                scr = scrp.tile([128, X], mybir.dt.float32,
                                tag="scr", name="scr")
                nc.vector.scalar_tensor_tensor(
                    out=scr[:], in0=gthj[:], scalar=1.0,
                    in1=xft[:, k, :], op0=MUL, op1=MUL,
                    accum_out=dt[:, j:j + 1])
                scr2 = scrp.tile([128, X], mybir.dt.float32,
                                 tag="scr2", name="scr2")
                nc.scalar.activation(
                    out=scr2[:], in_=gthj[:],
                    func=mybir.ActivationFunctionType.Square,
                    accum_out=nm[:, j:j + 1])
            nc.sync.dma_start(cdot[k], dt[:])
            nc.sync.dma_start(cnrm[k], nm[:])

    nc.compile()
    return nc


def _get_nc():
    if _nc_cache[0] is None:
        _nc_cache[0] = _build()
    return _nc_cache[0]


def _prep_inputs(x, x2y_w, y_neuron_age):
    x_flat = np.ascontiguousarray(np.asarray(x, np.float32).reshape(B, X))
    mask = (np.asarray(y_neuron_age, np.float32)[0] >= 1.0)
    xTb = np.ascontiguousarray(x_flat.T).astype(BF)
    W = np.asarray(x2y_w, np.float32)
    iota = np.ascontiguousarray(
        np.broadcast_to(np.arange(NREG * 8, dtype=np.float32),
                        (128, NREG * 8)))
    in_maps = []
    for c in range(N_CORES):
        sl = slice(c * YL, (c + 1) * YL)
        Wm = W[sl] * mask[sl, None]
        in_maps.append({
            "xTb": xTb,
            "xf": x_flat,
            "wTb": np.ascontiguousarray(Wm.T.astype(BF)),
            "wf": np.ascontiguousarray(W[sl]),
            "iota": iota,
        })
    return in_maps


def _merge(results, y2z_w):
    y2z = np.asarray(y2z_w, np.float32)
    idx_all, val_all = [], []
    for c in range(N_CORES):
        r = results[c]
        mi = r["cidx"].astype(np.int64)                      # [2,128,NCAND]
        p1 = np.asarray(r["cp1"], dtype=np.float32)[:, :, :NCAND]
        dt = r["cdot"].astype(np.float64)
        nm = r["cnrm"].astype(np.float64)
        idx_b = np.concatenate([mi[0], mi[1]], axis=0)       # [256, NCAND]
        p1_b = np.concatenate([p1[0], p1[1]], axis=0)
        dt_b = np.concatenate([dt[0], dt[1]], axis=0)
        nm_b = np.concatenate([nm[0], nm[1]], axis=0)
        with np.errstate(divide="ignore", invalid="ignore"):
            val = np.where(p1_b == 0.0, 0.0, dt_b / np.sqrt(nm_b))
        idx_all.append(c * YL + idx_b)
        val_all.append(val)
    idx = np.concatenate(idx_all, axis=1)
    val = np.concatenate(val_all, axis=1)
    order = np.argsort(idx, axis=1, kind="stable")
    idx_s = np.take_along_axis(idx, order, axis=1)
    val_s = np.take_along_axis(val, order, axis=1)
    best = val_s.argmax(axis=1)      # first max -> lowest global index on ties
    winner = idx_s[np.arange(B), best]
    output = y2z[:, winner].T.astype(np.float32)
    z_pre = np.argmax(output, axis=1).astype(np.int32)
    return output, z_pre


def _reference_fallback(x, z, x2y_w, y2z_w, y_neuron_age, test_cnt):
    """Numpy reference path for test_cnt != 0 (not exercised by the fixed
    setup_inputs, which always produces test_cnt == 0)."""
    xf = np.asarray(x, np.float32).reshape(B, -1)
    xf = xf / np.maximum(np.linalg.norm(xf, axis=1, keepdims=True), 1e-12)
    W = np.asarray(x2y_w, np.float32)
    w = W / np.maximum(np.linalg.norm(W, axis=1, keepdims=True), 1e-12)
    y_bu = xf @ w.T
    y_act = (np.asarray(y_neuron_age, np.float32) >= 1.0).astype(np.float32)
    y_pre = y_bu * y_act
    k = int(test_cnt) + 1
    idxs = np.argsort(-y_pre, axis=1, kind="stable")[:, :k]
    max_index = idxs[:, k - 1]
    y2z = np.asarray(y2z_w, np.float32)
    output = y2z[:, max_index].T.astype(np.float32)
    z_pre = np.argmax(output, axis=1).astype(np.int32)
    return output, z_pre


def kernel(x, z, x2y_w, y2z_w, y_neuron_age, test_cnt):
    if int(np.asarray(test_cnt)) != 0:
        return _reference_fallback(x, z, x2y_w, y2z_w, y_neuron_age, test_cnt)
    nc = _get_nc()
    in_maps = _prep_inputs(x, x2y_w, y_neuron_age)
    res = run_bass_kernel_spmd(nc, in_maps, list(range(N_CORES)))
    return _merge(res.results, y2z_w)
